# revision 16
# baseline (speedup 1.0000x reference)
"""GQA attention kernel for Trainium2, 8 NeuronCores.

Problem: resid [2, 2048, 1024], 16 Q heads / 8 KV groups, d_head 64, causal,
out = softmax(QK^T/8 + causal) V -> W_out + b_out.

Sharding: tensor-parallel over (batch x kv-group-pairs). Core c handles
batch b = c // 4 and kv groups {2*(c%4), 2*(c%4)+1} = 4 Q heads. Each core
computes its heads' attention and a partial output projection; the host sums
the 4 partials per batch element and adds b_out.

Per-core dataflow, all-bf16 operands (fp32 PSUM accumulation):
  - host passes resid[b].T (bf16) so the d_model contraction lands on
    partitions; weights pre-packed per core, bf16
  - per span (512 q): Q^T [256, S], K^T [128, S] projections; V [S, 2x65]
    with a ones column per group so the AV matmul produces sum-exp in row 64
  - scores transposed: S^T[k, q] = K @ Q^T; the two kv-groups use base
    partitions 0/64 so their matmuls row-pack the PE array and run
    concurrently; both land in one [128, 1024] two-bank PSUM tile
  - one [128, 1024] exp per k-tile on ScalarE (2-bank read amortizes the
    ~350-cycle activation pipe-fill); causal handled by q-start offset and
    an upper-triangular multiplicative mask on diagonal tiles (DVE, bf16)
  - heads processed in 2 passes per span so PSUM fits: 2 u-accumulator
    banks + 2x2 double-buffered score banks + 2 filler banks
  - projection / output-projection matmuls are emitted as "filler" units
    between attention iterations so the PE never idles (HAM stays warm)
  - normalize: sum-exp -> DVE reciprocal_approx_fast, Pool broadcast,
    DVE multiply -> z^T (bf16)
  - out_partial[s, d] = z^T.T @ W_out_stack, staged bf16, host upcasts+sums
"""

import sys

sys.path.insert(0, "/opt/trn_rl_repo")

import numpy as np
import ml_dtypes

import concourse.bass as bass
import concourse.mybir as mybir
import concourse.tile as tile
from concourse import bacc
from concourse.bass_utils import run_bass_kernel_spmd
from concourse.masks import make_upper_triangular

S = 2048          # seq len
D = 1024          # d_model
E = 64            # d_head
P = 128
NCHUNK = D // P   # 8 d_model chunks
SPAN = 512
NSPAN = S // SPAN
NKT = S // P      # 16 k tiles
F32 = mybir.dt.float32
BF16 = mybir.dt.bfloat16
EXP = mybir.ActivationFunctionType.Exp

LAST_RESULTS = None  # stashed BassKernelResults for the test harness
_CACHED_NC = None


def _build_program(debug=False):
    nc = bacc.Bacc("TRN2", target_bir_lowering=False, debug=False)
    dbg = {}

    def dbg_out(name, shape, dt):
        dbg[name] = nc.dram_tensor(name, shape, dt, kind="ExternalOutput")
        return dbg[name]

    rT_d = nc.dram_tensor("resid_t", [D, S], BF16, kind="ExternalInput")
    wkv_d = nc.dram_tensor("wkv", [P, 2048], BF16, kind="ExternalInput")
    wqo_d = nc.dram_tensor("wqo", [P, 4096], BF16, kind="ExternalInput")
    out_d = nc.dram_tensor("out", [S, D], BF16, kind="ExternalOutput")

    with tile.TileContext(nc) as tc:
        with (
            tc.tile_pool(name="persist", bufs=1) as pp,
            tc.tile_pool(name="exp", bufs=6) as ep,
            tc.tile_pool(name="norm", bufs=3) as mp,
            tc.tile_pool(name="ostage", bufs=3) as op,
            tc.tile_pool(name="ps_sc", bufs=2, space="PSUM") as ps_sc,
            tc.tile_pool(name="ps_u", bufs=2, space="PSUM") as ps_u,
            tc.tile_pool(name="ps_f", bufs=2, space="PSUM") as ps_f,
        ):
            # ---- PE warmup: a dependency-free matmul chain flips the HAM
            # clock gate to 8/8 (~3.4us of continuous PE busy) before the
            # first projection arrives, so nothing runs at the 1.2GHz cold
            # clock. Results land in a score-pool slot and are discarded. ----
            dummy = pp.tile([P, SPAN], BF16, tag="dummy")
            nc.gpsimd.memset(dummy[:], 0.0)
            warm = ps_sc.tile([P, 2 * SPAN], F32, tag="sc", name="warm")
            for i in range(26):
                nc.tensor.matmul(
                    warm[:, 0:SPAN], dummy[:, 0:P], dummy[:],
                    start=(i == 0), stop=(i == 25),
                )

            # ---- startup DMA: span-0 resid first on both HWDGE queues
            # (HBM landing order gates the first projections), packed
            # weights next; the scalar queue is then free for exps ----
            rT = [pp.tile([P, S], BF16, tag=f"rt{c}", name=f"rt{c}")
                  for c in range(NCHUNK)]
            wkv = pp.tile([P, 2048], BF16, tag="wkv")
            wqo = pp.tile([P, 4096], BF16, tag="wqo")
            for c in range(NCHUNK):
                eng = nc.scalar if c % 2 == 0 else nc.sync
                eng.dma_start(rT[c][:, 0:SPAN], rT_d[c * P:(c + 1) * P, 0:SPAN])
            nc.sync.dma_start(wkv[:], wkv_d[:, :])
            nc.scalar.dma_start(wqo[:], wqo_d[:, :])
            for c in range(NCHUNK):
                nc.sync.dma_start(rT[c][:, SPAN:S],
                                  rT_d[c * P:(c + 1) * P, SPAN:S])
            wk_sb = [wkv[:, c * 256:c * 256 + 128] for c in range(NCHUNK)]
            wv_sb = [wkv[:, c * 256 + 128:c * 256 + 256] for c in range(NCHUNK)]
            wq_sb = [wqo[:, c * 256:(c + 1) * 256] for c in range(NCHUNK)]
            wo_sb = [wqo[:, 2048 + c * D:2048 + (c + 1) * D] for c in range(2)]

            mask = pp.tile([P, P], BF16, tag="mask")
            make_upper_triangular(nc, mask[:], val=1.0, diag=True)

            qT = [pp.tile([P, S], BF16, tag=f"qt{e}", name=f"qt{e}")
                  for e in range(2)]
            kT = pp.tile([P, S], BF16, tag="kt")
            vaug = [pp.tile([P, 130], BF16, tag=f"va{k}", name=f"va{k}")
                    for k in range(NKT)]
            for k in range(NKT):
                nc.gpsimd.memset(vaug[k][:, 64:65], 1.0)
                nc.gpsimd.memset(vaug[k][:, 129:130], 1.0)
            # z^T per (span, pass): rows g*64..g*64+63 = head slot (g, i)
            zc = [[pp.tile([P, SPAN], BF16, tag=f"zc{sp}{i}", name=f"z{sp}{i}")
                   for i in range(2)] for sp in range(NSPAN)]

            # ---- filler units: projection + output-projection matmul
            # groups run between attention iterations on 2 spare PSUM banks
            # so the PE never goes idle while ScalarE works through exps ----
            filler = []
            op_filler = []

            def _chain_units(lhs_of, sp, dst, n_half=4):
                # an 8-chunk accumulation split into two pump units that
                # share one PSUM slot (finer PE interleave granularity)
                cell = {}

                def go_a():
                    acc = ps_f.tile([P, SPAN], F32, tag="f", name="pa")
                    cell['acc'] = acc
                    for c in range(n_half):
                        nc.tensor.matmul(
                            cell['acc'][:],
                            lhs_of(c),
                            rT[c][:, sp * SPAN:(sp + 1) * SPAN],
                            start=(c == 0),
                            stop=False,
                            skip_group_check=True,
                        )

                def go_b():
                    acc = cell['acc']
                    for c in range(n_half, NCHUNK):
                        nc.tensor.matmul(
                            acc[:],
                            lhs_of(c),
                            rT[c][:, sp * SPAN:(sp + 1) * SPAN],
                            start=False,
                            stop=(c == NCHUNK - 1),
                            skip_group_check=True,
                        )
                    nc.vector.tensor_copy(dst, acc[:])
                return [go_a, go_b]

            def q_proj_unit(sp, eblk):
                return _chain_units(
                    lambda c: wq_sb[c][:, eblk * P:(eblk + 1) * P], sp,
                    qT[eblk][:, sp * SPAN:(sp + 1) * SPAN])

            def k_proj_unit(sp):
                return _chain_units(
                    lambda c: wk_sb[c], sp,
                    kT[:, sp * SPAN:(sp + 1) * SPAN])

            def v_proj_unit(kt):
                def go():
                    acc = ps_f.tile([P, SPAN], F32, tag="f", name="vacc")
                    for c in range(NCHUNK):
                        nc.tensor.matmul(
                            acc[:, 0:128],
                            rT[c][:, kt * P:(kt + 1) * P],
                            wv_sb[c],
                            start=(c == 0),
                            stop=(c == NCHUNK - 1),
                        )
                    nc.vector.tensor_copy(vaug[kt][:, 0:64], acc[:, 0:64])
                    nc.vector.tensor_copy(vaug[kt][:, 65:129], acc[:, 64:128])
                return go

            def op_unit(sp, st):
                s0 = sp * SPAN + st * P
                cell = {}

                def go_dsp(dsp):
                    def go():
                        if dsp == 0:
                            cell['o'] = op.tile([P, D], BF16, tag="ost", name="osb")
                        o_sb = cell['o']
                        o_ps = ps_f.tile([P, SPAN], F32, tag="f", name="ops")
                        for ch in range(2):
                            nc.tensor.matmul(
                                o_ps[:],
                                zc[sp][ch][:, st * P:(st + 1) * P],
                                wo_sb[ch][:, dsp * SPAN:(dsp + 1) * SPAN],
                                start=(ch == 0),
                                stop=(ch == 1),
                            )
                        if sp == NSPAN - 1:
                            nc.scalar.activation(
                                o_sb[:, dsp * SPAN:(dsp + 1) * SPAN], o_ps[:],
                                mybir.ActivationFunctionType.Copy)
                        else:
                            nc.vector.tensor_copy(
                                o_sb[:, dsp * SPAN:(dsp + 1) * SPAN], o_ps[:])
                        if dsp == 1:
                            nc.sync.dma_start(out_d[s0:s0 + P, :], o_sb[:])
                    return go
                return [go_dsp(0), go_dsp(1)]

            def pump(n, ops_ok=False):
                for _ in range(n):
                    if filler:
                        filler.pop(0)()
                    elif ops_ok and op_filler:
                        op_filler.pop(0)()
                    else:
                        break

            def proj_units(sp):
                u = k_proj_unit(sp)
                u += [v_proj_unit(kt) for kt in range(4 * sp, 4 * sp + 4)]
                u += q_proj_unit(sp, 0) + q_proj_unit(sp, 1)
                return u

            # span 0 projections run up front; K then Q0 first so the
            # pass-0 score/exp chain starts as early as possible
            for f in (k_proj_unit(0) + q_proj_unit(0, 0)
                      + [v_proj_unit(kt) for kt in range(4)]
                      + q_proj_unit(0, 1)):
                f()

            for sp in range(NSPAN):
                q0 = sp * SPAN
                nkt = (q0 + SPAN) // P
                if sp + 1 < NSPAN:
                    filler.extend(proj_units(sp + 1))
                for ip in range(2):
                    u_ps = [ps_u.tile([65, SPAN], F32, tag="u", name=f"u{g}")
                            for g in range(2)]

                    def emit_av(b):
                        kt_, off_, w_, e_ = b
                        for g in range(2):
                            nc.tensor.matmul(
                                u_ps[g][0:65, off_:off_ + w_],
                                vaug[kt_][:, g * 65:(g + 1) * 65],
                                e_[:, g * 512 + off_:g * 512 + off_ + w_],
                                start=(kt_ == 0),
                                stop=(kt_ == nkt - 1),
                                skip_group_check=True,
                            )

                    pending = []
                    for kt in range(nkt):
                        k0 = kt * P
                        off = max(k0 - q0, 0)
                        w = SPAN - off
                        s_ps = ps_sc.tile([P, 2 * SPAN], F32, tag="sc",
                                          name="sps")
                        for g in range(2):
                            nc.tensor.matmul(
                                s_ps[:, g * 512 + off:g * 512 + off + w],
                                kT[g * 64:(g + 1) * 64, k0:k0 + P],
                                qT[ip][g * 64:(g + 1) * 64,
                                       q0 + off:q0 + off + w],
                                start=True,
                                stop=True,
                            )
                        e_sb = ep.tile([P, 2 * SPAN], BF16, tag="e", name="e")
                        nc.scalar.activation(
                            e_sb[:, off:2 * SPAN], s_ps[:, off:2 * SPAN],
                            EXP, scale=0.125,
                        )
                        if k0 >= q0:  # diagonal tile -> causal mask
                            for g in range(2):
                                nc.vector.tensor_mul(
                                    e_sb[:, g * 512 + off:g * 512 + off + P],
                                    e_sb[:, g * 512 + off:g * 512 + off + P],
                                    mask[:],
                                )
                        if debug and (sp, ip, kt) in ((0, 0, 0), (1, 0, 2)):
                            t = dbg_out(f"d_e_{sp}_{ip}_{kt}",
                                        [P, 2 * SPAN], BF16)
                            nc.sync.dma_start(t[:], e_sb[:])
                        pending.append((kt, off, w, e_sb))
                        pump(1, ops_ok=(sp == NSPAN - 1))
                        if len(pending) > 2:
                            emit_av(pending.pop(0))
                    for b in pending:
                        emit_av(b)
                    if debug and sp == 0 and ip == 0:
                        for g in range(2):
                            us = pp.tile([65, SPAN], F32, tag=f"dbgu{g}")
                            nc.vector.tensor_copy(us[:], u_ps[g][:])
                            t = dbg_out(f"d_u_{g}", [65, SPAN], F32)
                            nc.sync.dma_start(t[:], us[:])

                    # normalize this pass -> z^T slabs
                    last = (sp == NSPAN - 1 and ip == 1)
                    for g in range(2):
                        # standard-op copy remaps partition 64 -> 0; the
                        # custom-DVE reciprocal needs lane-aligned operands
                        row = mp.tile([1, SPAN], F32, tag="row", name="row")
                        if last and g == 0:
                            nc.scalar.activation(
                                row[:], u_ps[g][64:65, :],
                                mybir.ActivationFunctionType.Copy)
                        else:
                            nc.vector.tensor_copy(row[:], u_ps[g][64:65, :])
                        rec = mp.tile([1, SPAN], F32, tag="rec", name="rec")
                        nc.vector.reciprocal_approx_fast(rec[:], row[:])
                        bc = mp.tile([64, SPAN], F32, tag="bc", name="bc")
                        nc.gpsimd.partition_broadcast(bc[:], rec[:])
                        nc.vector.tensor_mul(
                            zc[sp][ip][g * 64:(g + 1) * 64, :],
                            u_ps[g][0:64, :],
                            bc[:],
                        )
                        if debug and sp == 0 and ip == 0:
                            rs = pp.tile([1, SPAN], F32, tag=f"dbgr{g}")
                            nc.vector.tensor_copy(rs[:], rec[:])
                            t = dbg_out(f"d_rec_{g}", [1, SPAN], F32)
                            nc.sync.dma_start(t[:], rs[:])
                if debug and sp == 0:
                    for i in range(2):
                        t = dbg_out(f"d_zc_{i}", [P, SPAN], BF16)
                        nc.sync.dma_start(t[:], zc[0][i][:])
                for st in range(4):
                    op_filler.extend(op_unit(sp, st))
            pump(len(filler) + len(op_filler), ops_ok=True)
            if debug:
                for nm, ap in (("d_mask", mask), ("d_kT", kT),
                               ("d_qT0", qT[0]), ("d_qT1", qT[1]),
                               ("d_va0", vaug[0]), ("d_va5", vaug[5])):
                    t = dbg_out(nm, list(ap.shape), BF16)
                    nc.sync.dma_start(t[:], ap[:])

    nc.finalize()
    return nc


def _pack_weights(wq4, wk2, wv2, wo4):
    """Pack per-core weight slices into the two bf16 tensors the kernel
    expects: wkv [128, 2048] = per chunk c cols [c*256, c*256+256) = wk|wv;
    wqo [128, 4096] = per chunk wq (256 cols), then wo chunk r at
    2048 + r*1024."""
    bf16 = ml_dtypes.bfloat16
    wq = np.ascontiguousarray(wq4.reshape(D, 256))
    wk = np.ascontiguousarray(wk2.reshape(D, 128))
    wv = np.ascontiguousarray(wv2.reshape(D, 128))
    wo = np.ascontiguousarray(wo4.transpose(1, 0, 2).reshape(256, D))
    wkv = np.zeros((P, 2048), np.float32)
    wqo = np.zeros((P, 4096), np.float32)
    for c in range(NCHUNK):
        r = slice(c * P, (c + 1) * P)
        wkv[:, c * 256:c * 256 + 128] = wk[r, :]
        wkv[:, c * 256 + 128:c * 256 + 256] = wv[r, :]
        wqo[:, c * 256:(c + 1) * 256] = wq[r, :]
    wqo[:, 2048:3072] = wo[0:128, :]
    wqo[:, 3072:4096] = wo[128:256, :]
    return wkv.astype(bf16), wqo.astype(bf16)


def kernel(resid, W_Q, W_K, W_V, W_out, b_out):
    global LAST_RESULTS, _CACHED_NC
    resid = np.asarray(resid, np.float32)
    W_Q = np.asarray(W_Q, np.float32)
    W_K = np.asarray(W_K, np.float32)
    W_V = np.asarray(W_V, np.float32)
    W_out = np.asarray(W_out, np.float32)
    b_out = np.asarray(b_out, np.float32)
    bf16 = ml_dtypes.bfloat16

    if _CACHED_NC is None:
        _CACHED_NC = _build_program()
    nc = _CACHED_NC

    residT = [np.ascontiguousarray(resid[b].T).astype(bf16) for b in range(2)]
    in_maps = []
    for c in range(8):
        b, q = c // 4, c % 4
        # interleaved head order [h0, h2, h1, h3]: storage slot (g, i) holds
        # local head 2g+i -> qT[i]/zc[i] rows g*64 (see _build_program)
        heads = [4 * q, 4 * q + 2, 4 * q + 1, 4 * q + 3]
        groups = [2 * q, 2 * q + 1]
        wkv, wqo = _pack_weights(W_Q[:, heads, :], W_K[:, groups, :],
                                 W_V[:, groups, :], W_out[:, heads, :])
        in_maps.append({
            "resid_t": residT[b],
            "wkv": wkv,
            "wqo": wqo,
        })

    res = run_bass_kernel_spmd(nc, in_maps, core_ids=list(range(8)))
    LAST_RESULTS = res

    out = np.zeros((2, S, D), np.float32)
    for c in range(8):
        out[c // 4] += np.asarray(res.results[c]["out"], np.float32)
    out += b_out
    return out


# revision 17
# speedup vs baseline: 1.0200x; 1.0200x over previous
"""GQA attention kernel for Trainium2, 8 NeuronCores.

Problem: resid [2, 2048, 1024], 16 Q heads / 8 KV groups, d_head 64, causal,
out = softmax(QK^T/8 + causal) V -> W_out + b_out.

Sharding: tensor-parallel over (batch x kv-group-pairs). Core c handles
batch b = c // 4 and kv groups {2*(c%4), 2*(c%4)+1} = 4 Q heads. Each core
computes its heads' attention and a partial output projection; the host sums
the 4 partials per batch element and adds b_out.

Per-core dataflow, all-bf16 operands (fp32 PSUM accumulation):
  - host passes resid[b].T (bf16) so the d_model contraction lands on
    partitions; weights pre-packed per core, bf16
  - per span (512 q): Q^T [256, S], K^T [128, S] projections; V [S, 2x65]
    with a ones column per group so the AV matmul produces sum-exp in row 64
  - scores transposed: S^T[k, q] = K @ Q^T; the two kv-groups use base
    partitions 0/64 so their matmuls row-pack the PE array and run
    concurrently; both land in one [128, 1024] two-bank PSUM tile
  - one [128, 1024] exp per k-tile on ScalarE (2-bank read amortizes the
    ~350-cycle activation pipe-fill); causal handled by q-start offset and
    an upper-triangular multiplicative mask on diagonal tiles (DVE, bf16)
  - heads processed in 2 passes per span so PSUM fits: 2 u-accumulator
    banks + 2x2 double-buffered score banks + 2 filler banks
  - projection / output-projection matmuls are emitted as "filler" units
    between attention iterations so the PE never idles (HAM stays warm)
  - normalize: sum-exp -> DVE reciprocal_approx_fast, Pool broadcast,
    DVE multiply -> z^T (bf16)
  - out_partial[s, d] = z^T.T @ W_out_stack, staged bf16, host upcasts+sums
"""

import sys

sys.path.insert(0, "/opt/trn_rl_repo")

import numpy as np
import ml_dtypes

import concourse.bass as bass
import concourse.mybir as mybir
import concourse.tile as tile
from concourse import bacc
from concourse.bass_utils import run_bass_kernel_spmd
from concourse.masks import make_upper_triangular

S = 2048          # seq len
D = 1024          # d_model
E = 64            # d_head
P = 128
NCHUNK = D // P   # 8 d_model chunks
SPAN = 512
NSPAN = S // SPAN
NKT = S // P      # 16 k tiles
F32 = mybir.dt.float32
BF16 = mybir.dt.bfloat16
EXP = mybir.ActivationFunctionType.Exp

LAST_RESULTS = None  # stashed BassKernelResults for the test harness
_CACHED_NC = None


def _build_program(debug=False):
    nc = bacc.Bacc("TRN2", target_bir_lowering=False, debug=False)
    dbg = {}

    def dbg_out(name, shape, dt):
        dbg[name] = nc.dram_tensor(name, shape, dt, kind="ExternalOutput")
        return dbg[name]

    rT_d = nc.dram_tensor("resid_t", [D, S], BF16, kind="ExternalInput")
    wk_d = nc.dram_tensor("wk8", [P, 1024], BF16, kind="ExternalInput")
    wv_d = nc.dram_tensor("wv8", [P, 1024], BF16, kind="ExternalInput")
    wqo_d = nc.dram_tensor("wqo", [P, 4096], BF16, kind="ExternalInput")
    out_d = nc.dram_tensor("out", [S, D], BF16, kind="ExternalOutput")

    with tile.TileContext(nc) as tc:
        with (
            tc.tile_pool(name="persist", bufs=1) as pp,
            tc.tile_pool(name="exp", bufs=6) as ep,
            tc.tile_pool(name="norm", bufs=3) as mp,
            tc.tile_pool(name="ostage", bufs=3) as op,
            tc.tile_pool(name="ps_sc", bufs=2, space="PSUM") as ps_sc,
            tc.tile_pool(name="ps_u", bufs=2, space="PSUM") as ps_u,
            tc.tile_pool(name="ps_f", bufs=2, space="PSUM") as ps_f,
        ):
            # ---- PE warmup: a dependency-free matmul chain flips the HAM
            # clock gate to 8/8 (~3.4us of continuous PE busy) before the
            # first projection arrives, so nothing runs at the 1.2GHz cold
            # clock. Results land in a score-pool slot and are discarded. ----
            dummy = pp.tile([P, SPAN], BF16, tag="dummy")
            nc.gpsimd.memset(dummy[:], 0.0)
            warm = ps_sc.tile([P, 2 * SPAN], F32, tag="sc", name="warm")
            for i in range(26):
                nc.tensor.matmul(
                    warm[:, 0:SPAN], dummy[:, 0:P], dummy[:],
                    start=(i == 0), stop=(i == 25),
                )

            # ---- startup DMA: span-0 resid first on both HWDGE queues
            # (HBM landing order gates the first projections), packed
            # weights next; the scalar queue is then free for exps ----
            rT = [pp.tile([P, S], BF16, tag=f"rt{c}", name=f"rt{c}")
                  for c in range(NCHUNK)]
            wkt = pp.tile([P, 1024], BF16, tag="wkt")
            wvt = pp.tile([P, 1024], BF16, tag="wvt")
            wqo = pp.tile([P, 4096], BF16, tag="wqo")
            nc.sync.dma_start(wkt[:], wk_d[:, :])
            for c in range(NCHUNK):
                eng = nc.scalar if c % 2 == 0 else nc.sync
                eng.dma_start(rT[c][:, 0:SPAN], rT_d[c * P:(c + 1) * P, 0:SPAN])
            nc.sync.dma_start(wvt[:], wv_d[:, :])
            nc.scalar.dma_start(wqo[:], wqo_d[:, :])
            for c in range(NCHUNK):
                nc.sync.dma_start(rT[c][:, SPAN:S],
                                  rT_d[c * P:(c + 1) * P, SPAN:S])
            wk_sb = [wkt[:, c * 128:(c + 1) * 128] for c in range(NCHUNK)]
            wv_sb = [wvt[:, c * 128:(c + 1) * 128] for c in range(NCHUNK)]
            wq_sb = [wqo[:, c * 256:(c + 1) * 256] for c in range(NCHUNK)]
            wo_sb = [wqo[:, 2048 + c * D:2048 + (c + 1) * D] for c in range(2)]

            mask = pp.tile([P, P], BF16, tag="mask")
            make_upper_triangular(nc, mask[:], val=1.0, diag=True)

            qT = [pp.tile([P, S], BF16, tag=f"qt{e}", name=f"qt{e}")
                  for e in range(2)]
            kT = pp.tile([P, S], BF16, tag="kt")
            vaug = [pp.tile([P, 130], BF16, tag=f"va{k}", name=f"va{k}")
                    for k in range(NKT)]
            for k in range(NKT):
                nc.gpsimd.memset(vaug[k][:, 64:65], 1.0)
                nc.gpsimd.memset(vaug[k][:, 129:130], 1.0)
            # z^T per (span, pass): rows g*64..g*64+63 = head slot (g, i)
            zc = [[pp.tile([P, SPAN], BF16, tag=f"zc{sp}{i}", name=f"z{sp}{i}")
                   for i in range(2)] for sp in range(NSPAN)]

            # ---- filler units: projection + output-projection matmul
            # groups run between attention iterations on 2 spare PSUM banks
            # so the PE never goes idle while ScalarE works through exps ----
            filler = []
            op_filler = []

            def _chain_units(lhs_of, sp, dst, n_half=4):
                # an 8-chunk accumulation split into two pump units that
                # share one PSUM slot (finer PE interleave granularity)
                cell = {}

                def go_a():
                    acc = ps_f.tile([P, SPAN], F32, tag="f", name="pa")
                    cell['acc'] = acc
                    for c in range(n_half):
                        nc.tensor.matmul(
                            cell['acc'][:],
                            lhs_of(c),
                            rT[c][:, sp * SPAN:(sp + 1) * SPAN],
                            start=(c == 0),
                            stop=False,
                            skip_group_check=True,
                        )

                def go_b():
                    acc = cell['acc']
                    for c in range(n_half, NCHUNK):
                        nc.tensor.matmul(
                            acc[:],
                            lhs_of(c),
                            rT[c][:, sp * SPAN:(sp + 1) * SPAN],
                            start=False,
                            stop=(c == NCHUNK - 1),
                            skip_group_check=True,
                        )
                    nc.vector.tensor_copy(dst, acc[:])
                return [go_a, go_b]

            def q_proj_unit(sp, eblk):
                return _chain_units(
                    lambda c: wq_sb[c][:, eblk * P:(eblk + 1) * P], sp,
                    qT[eblk][:, sp * SPAN:(sp + 1) * SPAN])

            def k_proj_unit(sp):
                return _chain_units(
                    lambda c: wk_sb[c], sp,
                    kT[:, sp * SPAN:(sp + 1) * SPAN])

            def v_proj_unit(kt):
                def go():
                    acc = ps_f.tile([P, SPAN], F32, tag="f", name="vacc")
                    for c in range(NCHUNK):
                        nc.tensor.matmul(
                            acc[:, 0:128],
                            rT[c][:, kt * P:(kt + 1) * P],
                            wv_sb[c],
                            start=(c == 0),
                            stop=(c == NCHUNK - 1),
                        )
                    nc.vector.tensor_copy(vaug[kt][:, 0:64], acc[:, 0:64])
                    nc.vector.tensor_copy(vaug[kt][:, 65:129], acc[:, 64:128])
                return go

            def op_unit(sp, st):
                s0 = sp * SPAN + st * P
                cell = {}

                def go_dsp(dsp):
                    def go():
                        if dsp == 0:
                            cell['o'] = op.tile([P, D], BF16, tag="ost", name="osb")
                        o_sb = cell['o']
                        o_ps = ps_f.tile([P, SPAN], F32, tag="f", name="ops")
                        for ch in range(2):
                            nc.tensor.matmul(
                                o_ps[:],
                                zc[sp][ch][:, st * P:(st + 1) * P],
                                wo_sb[ch][:, dsp * SPAN:(dsp + 1) * SPAN],
                                start=(ch == 0),
                                stop=(ch == 1),
                            )
                        if sp == NSPAN - 1:
                            nc.scalar.activation(
                                o_sb[:, dsp * SPAN:(dsp + 1) * SPAN], o_ps[:],
                                mybir.ActivationFunctionType.Copy)
                        else:
                            nc.vector.tensor_copy(
                                o_sb[:, dsp * SPAN:(dsp + 1) * SPAN], o_ps[:])
                        if dsp == 1:
                            nc.sync.dma_start(out_d[s0:s0 + P, :], o_sb[:])
                    return go
                return [go_dsp(0), go_dsp(1)]

            def pump(n, ops_ok=False):
                for _ in range(n):
                    if filler:
                        filler.pop(0)()
                    elif ops_ok and op_filler:
                        op_filler.pop(0)()
                    else:
                        break

            def proj_units(sp):
                u = k_proj_unit(sp)
                u += [v_proj_unit(kt) for kt in range(4 * sp, 4 * sp + 4)]
                u += q_proj_unit(sp, 0) + q_proj_unit(sp, 1)
                return u

            # span 0 projections run up front; K then Q0 first so the
            # pass-0 score/exp chain starts as early as possible
            for f in (k_proj_unit(0) + q_proj_unit(0, 0)
                      + [v_proj_unit(kt) for kt in range(4)]
                      + q_proj_unit(0, 1)):
                f()

            for sp in range(NSPAN):
                q0 = sp * SPAN
                nkt = (q0 + SPAN) // P
                if sp + 1 < NSPAN:
                    filler.extend(proj_units(sp + 1))
                for ip in range(2):
                    u_ps = [ps_u.tile([65, SPAN], F32, tag="u", name=f"u{g}")
                            for g in range(2)]

                    def emit_av(b):
                        kt_, off_, w_, e_ = b
                        for g in range(2):
                            nc.tensor.matmul(
                                u_ps[g][0:65, off_:off_ + w_],
                                vaug[kt_][:, g * 65:(g + 1) * 65],
                                e_[:, g * 512 + off_:g * 512 + off_ + w_],
                                start=(kt_ == 0),
                                stop=(kt_ == nkt - 1),
                                skip_group_check=True,
                            )

                    pending = []
                    for kt in range(nkt):
                        k0 = kt * P
                        off = max(k0 - q0, 0)
                        w = SPAN - off
                        s_ps = ps_sc.tile([P, 2 * SPAN], F32, tag="sc",
                                          name="sps")
                        for g in range(2):
                            nc.tensor.matmul(
                                s_ps[:, g * 512 + off:g * 512 + off + w],
                                kT[g * 64:(g + 1) * 64, k0:k0 + P],
                                qT[ip][g * 64:(g + 1) * 64,
                                       q0 + off:q0 + off + w],
                                start=True,
                                stop=True,
                            )
                        e_sb = ep.tile([P, 2 * SPAN], BF16, tag="e", name="e")
                        nc.scalar.activation(
                            e_sb[:, off:2 * SPAN], s_ps[:, off:2 * SPAN],
                            EXP, scale=0.125,
                        )
                        if k0 >= q0:  # diagonal tile -> causal mask
                            for g in range(2):
                                nc.vector.tensor_mul(
                                    e_sb[:, g * 512 + off:g * 512 + off + P],
                                    e_sb[:, g * 512 + off:g * 512 + off + P],
                                    mask[:],
                                )
                        if debug and (sp, ip, kt) in ((0, 0, 0), (1, 0, 2)):
                            t = dbg_out(f"d_e_{sp}_{ip}_{kt}",
                                        [P, 2 * SPAN], BF16)
                            nc.sync.dma_start(t[:], e_sb[:])
                        pending.append((kt, off, w, e_sb))
                        pump(1, ops_ok=(sp == NSPAN - 1))
                        if len(pending) > 2:
                            emit_av(pending.pop(0))
                    for b in pending:
                        emit_av(b)
                    if debug and sp == 0 and ip == 0:
                        for g in range(2):
                            us = pp.tile([65, SPAN], F32, tag=f"dbgu{g}")
                            nc.vector.tensor_copy(us[:], u_ps[g][:])
                            t = dbg_out(f"d_u_{g}", [65, SPAN], F32)
                            nc.sync.dma_start(t[:], us[:])

                    # normalize this pass -> z^T slabs
                    last = (sp == NSPAN - 1 and ip == 1)
                    for g in range(2):
                        # standard-op copy remaps partition 64 -> 0; the
                        # custom-DVE reciprocal needs lane-aligned operands
                        row = mp.tile([1, SPAN], F32, tag="row", name="row")
                        if last and g == 0:
                            nc.scalar.activation(
                                row[:], u_ps[g][64:65, :],
                                mybir.ActivationFunctionType.Copy)
                        else:
                            nc.vector.tensor_copy(row[:], u_ps[g][64:65, :])
                        rec = mp.tile([1, SPAN], F32, tag="rec", name="rec")
                        nc.vector.reciprocal_approx_fast(rec[:], row[:])
                        bc = mp.tile([64, SPAN], F32, tag="bc", name="bc")
                        nc.gpsimd.partition_broadcast(bc[:], rec[:])
                        nc.vector.tensor_mul(
                            zc[sp][ip][g * 64:(g + 1) * 64, :],
                            u_ps[g][0:64, :],
                            bc[:],
                        )
                        if debug and sp == 0 and ip == 0:
                            rs = pp.tile([1, SPAN], F32, tag=f"dbgr{g}")
                            nc.vector.tensor_copy(rs[:], rec[:])
                            t = dbg_out(f"d_rec_{g}", [1, SPAN], F32)
                            nc.sync.dma_start(t[:], rs[:])
                if debug and sp == 0:
                    for i in range(2):
                        t = dbg_out(f"d_zc_{i}", [P, SPAN], BF16)
                        nc.sync.dma_start(t[:], zc[0][i][:])
                for st in range(4):
                    op_filler.extend(op_unit(sp, st))
            pump(len(filler) + len(op_filler), ops_ok=True)
            if debug:
                for nm, ap in (("d_mask", mask), ("d_kT", kT),
                               ("d_qT0", qT[0]), ("d_qT1", qT[1]),
                               ("d_va0", vaug[0]), ("d_va5", vaug[5])):
                    t = dbg_out(nm, list(ap.shape), BF16)
                    nc.sync.dma_start(t[:], ap[:])

    nc.finalize()
    return nc


def _pack_weights(wq4, wk2, wv2, wo4):
    """Pack per-core weight slices into the two bf16 tensors the kernel
    expects: wkv [128, 2048] = per chunk c cols [c*256, c*256+256) = wk|wv;
    wqo [128, 4096] = per chunk wq (256 cols), then wo chunk r at
    2048 + r*1024."""
    bf16 = ml_dtypes.bfloat16
    wq = np.ascontiguousarray(wq4.reshape(D, 256))
    wk = np.ascontiguousarray(wk2.reshape(D, 128))
    wv = np.ascontiguousarray(wv2.reshape(D, 128))
    wo = np.ascontiguousarray(wo4.transpose(1, 0, 2).reshape(256, D))
    wk8 = np.zeros((P, 1024), np.float32)
    wv8 = np.zeros((P, 1024), np.float32)
    wqo = np.zeros((P, 4096), np.float32)
    for c in range(NCHUNK):
        r = slice(c * P, (c + 1) * P)
        wk8[:, c * 128:(c + 1) * 128] = wk[r, :]
        wv8[:, c * 128:(c + 1) * 128] = wv[r, :]
        wqo[:, c * 256:(c + 1) * 256] = wq[r, :]
    wqo[:, 2048:3072] = wo[0:128, :]
    wqo[:, 3072:4096] = wo[128:256, :]
    return wk8.astype(bf16), wv8.astype(bf16), wqo.astype(bf16)


def kernel(resid, W_Q, W_K, W_V, W_out, b_out):
    global LAST_RESULTS, _CACHED_NC
    resid = np.asarray(resid, np.float32)
    W_Q = np.asarray(W_Q, np.float32)
    W_K = np.asarray(W_K, np.float32)
    W_V = np.asarray(W_V, np.float32)
    W_out = np.asarray(W_out, np.float32)
    b_out = np.asarray(b_out, np.float32)
    bf16 = ml_dtypes.bfloat16

    if _CACHED_NC is None:
        _CACHED_NC = _build_program()
    nc = _CACHED_NC

    residT = [np.ascontiguousarray(resid[b].T).astype(bf16) for b in range(2)]
    in_maps = []
    for c in range(8):
        b, q = c // 4, c % 4
        # interleaved head order [h0, h2, h1, h3]: storage slot (g, i) holds
        # local head 2g+i -> qT[i]/zc[i] rows g*64 (see _build_program)
        heads = [4 * q, 4 * q + 2, 4 * q + 1, 4 * q + 3]
        groups = [2 * q, 2 * q + 1]
        wk8, wv8, wqo = _pack_weights(W_Q[:, heads, :], W_K[:, groups, :],
                                      W_V[:, groups, :], W_out[:, heads, :])
        in_maps.append({
            "resid_t": residT[b],
            "wk8": wk8,
            "wv8": wv8,
            "wqo": wqo,
        })

    res = run_bass_kernel_spmd(nc, in_maps, core_ids=list(range(8)))
    LAST_RESULTS = res

    out = np.zeros((2, S, D), np.float32)
    for c in range(8):
        out[c // 4] += np.asarray(res.results[c]["out"], np.float32)
    out += b_out
    return out
